# revision 9
# baseline (speedup 1.0000x reference)
"""AntiAliasActivation (UpSample2x -> SnakeBeta -> DownSample2x) on 8 TRN2 NeuronCores.

Self-contained Trainium Bass kernel. Sharding: data-parallel over batch
(16 batches -> 2 per core); no cross-core communication.

Math (validated vs reference to 2e-7 in check_math.py):
  polyphase upsample:  P[v] = x[clamp(v-3)],
      ye[u] = sum_a g[a]  * P[u+a],    g[a]  = 2*uf[11-2a]
      yo[u] = sum_a go[a] * P[u+1+a],  go[a] = 2*uf[10-2a]
  snake via cos:  z = y + Cb*sin(Ca*y)^2 = (y - (Cb/2)*cos(2*Ca*y)) + Cb/2
      w = y - (Cb/2)*cos(2Ca*y).  ACT Sin is only valid on |arg|<~3.2 (no HW
      range reduction), so reduce explicitly: q = (Ca/pi)*y + 1/4 (= v/2pi for
      v = 2Ca*y + pi/2), k = round(q) via the 1.5*2^23 magic-number trick,
      d = q - k in [-1/2, 1/2], then cos(2Ca*y) = sin(2*pi*d).
  downsample on w-phases (the +Cb/2 constant folds into the output since the
  12 down taps always sum to sum(df) even at clamped edges):
      A_arr = [we0]*2 + we + [wo_last]*3   (len T+5)
      B_arr = [we0]*3 + wo + [wo_last]*2
      out[t] = sum_c h[c]*A_arr[t+c] + g2[c]*B_arr[t+c] + (Cb/2)*sum(df)
      h[c] = df[2c+1], g2[c] = df[2c]
  All FIR taps run on the TensorEngine as scaled-identity matmuls with PSUM
  accumulation (shifted windows of row-major SBUF tiles are the moving operand).
"""

import math
import sys
from contextlib import ExitStack

import ml_dtypes
import numpy as np

sys.path.insert(0, "/opt/trn_rl_repo")

import concourse.bass as bass  # noqa: E402
import concourse.bacc as bacc  # noqa: E402
import concourse.tile as tile  # noqa: E402
from concourse import mybir  # noqa: E402
from concourse.bass_utils import run_bass_kernel_spmd  # noqa: E402

F32 = mybir.dt.float32
BF16 = mybir.dt.bfloat16
SIN = mybir.ActivationFunctionType.Sin
MAGIC = 1.5 * 2.0 ** 23  # fp32 round-to-nearest via add/sub

B, C, T = 16, 512, 4096
NCORES = 8
BPC = B // NCORES              # batches per core = 2
RPC = BPC * C                  # rows per core = 1024
NT = RPC // 128                # row-tiles per core = 8
CH = 512                       # matmul moving free dim (one PSUM bank)
NCH = T // CH                  # column chunks per row-tile = 8

_CACHE = {}


def build_bass():
    nc = bacc.Bacc("TRN2", target_bir_lowering=False, debug=False, num_devices=NCORES)

    x_d = nc.dram_tensor("x", [RPC, T], F32, kind="ExternalInput").ap()
    w_d = nc.dram_tensor("wmats", [128, 24 * 128], BF16, kind="ExternalInput").ap()
    s1_d = nc.dram_tensor("s1", [128, NT], F32, kind="ExternalInput").ap()
    ncbh_d = nc.dram_tensor("ncbh", [128, NT], F32, kind="ExternalInput").ap()
    cbs_d = nc.dram_tensor("cbs", [128, NT], F32, kind="ExternalInput").ap()
    out_d = nc.dram_tensor("out", [RPC, T], F32, kind="ExternalOutput").ap()

    mult = mybir.AluOpType.mult
    add = mybir.AluOpType.add

    with tile.TileContext(nc) as tc, ExitStack() as ctx:
        singles = ctx.enter_context(tc.tile_pool(name="singles", bufs=1))
        xpool = ctx.enter_context(tc.tile_pool(name="xpool", bufs=2))
        wpool = ctx.enter_context(tc.tile_pool(name="wpool", bufs=2))
        tpool = ctx.enter_context(tc.tile_pool(name="tpool", bufs=3))
        opool = ctx.enter_context(tc.tile_pool(name="opool", bufs=2))
        psum = ctx.enter_context(tc.tile_pool(name="psum", bufs=2, space="PSUM"))

        wsb = singles.tile([128, 24 * 128], BF16)
        nc.sync.dma_start(wsb[:], w_d[:])
        s1 = singles.tile([128, NT], F32)
        nc.sync.dma_start(s1[:], s1_d[:])
        ncbh = singles.tile([128, NT], F32)
        nc.sync.dma_start(ncbh[:], ncbh_d[:])
        cbs = singles.tile([128, NT], F32)
        nc.sync.dma_start(cbs[:], cbs_d[:])


        def W(i):
            return wsb[:, i * 128:(i + 1) * 128]

        for rt in range(NT):
            rows = slice(rt * 128, (rt + 1) * 128)
            xp = xpool.tile([128, T + 6], BF16, tag="xp")
            nc.gpsimd.dma_start(xp[:, 3:3 + T], x_d[rows, :])
            for k in range(3):
                nc.vector.tensor_copy(xp[:, k:k + 1], xp[:, 3:4])
                nc.vector.tensor_copy(xp[:, T + 3 + k:T + 4 + k], xp[:, T + 2:T + 3])

            we = wpool.tile([128, T + 5], BF16, tag="we")
            wo = wpool.tile([128, T + 5], BF16, tag="wo")
            for chx in range(NCH):
                off = chx * CH
                ye = psum.tile([128, CH], F32, tag="ye")
                for a in range(6):
                    nc.tensor.matmul(
                        ye[:], lhsT=W(a),
                        rhs=xp[:, off + a:off + a + CH],
                        start=(a == 0), stop=(a == 5))
                qe = tpool.tile([128, CH], F32, tag="qe")
                nc.vector.tensor_scalar(qe[:], ye[:], s1[:, rt:rt + 1], 0.25,
                                        mult, add)
                ke = tpool.tile([128, CH], F32, tag="ke")
                nc.vector.tensor_scalar(ke[:], qe[:], MAGIC, -MAGIC, add, add)
                de = tpool.tile([128, CH], F32, tag="de")
                nc.vector.tensor_sub(de[:], qe[:], ke[:])
                ce = tpool.tile([128, CH], BF16, tag="ce")
                nc.scalar.activation(ce[:], de[:], SIN, scale=2 * math.pi)
                nc.vector.scalar_tensor_tensor(
                    out=we[:, 2 + off:2 + off + CH], in0=ce[:],
                    scalar=ncbh[:, rt:rt + 1], in1=ye[:], op0=mult, op1=add)

                yo = psum.tile([128, CH], F32, tag="yo")
                for a in range(6):
                    nc.tensor.matmul(
                        yo[:], lhsT=W(6 + a),
                        rhs=xp[:, off + 1 + a:off + 1 + a + CH],
                        start=(a == 0), stop=(a == 5))
                qo = tpool.tile([128, CH], F32, tag="qo")
                nc.vector.tensor_scalar(qo[:], yo[:], s1[:, rt:rt + 1], 0.25,
                                        mult, add)
                ko = tpool.tile([128, CH], F32, tag="ko")
                nc.vector.tensor_scalar(ko[:], qo[:], MAGIC, -MAGIC, add, add)
                do = tpool.tile([128, CH], F32, tag="do")
                nc.vector.tensor_sub(do[:], qo[:], ko[:])
                co = tpool.tile([128, CH], BF16, tag="co")
                nc.scalar.activation(co[:], do[:], SIN, scale=2 * math.pi)
                nc.vector.scalar_tensor_tensor(
                    out=wo[:, 3 + off:3 + off + CH], in0=co[:],
                    scalar=ncbh[:, rt:rt + 1], in1=yo[:], op0=mult, op1=add)

            # edge pads at the w level (replicate semantics of the reference)
            nc.vector.tensor_copy(we[:, 0:1], we[:, 2:3])
            nc.vector.tensor_copy(we[:, 1:2], we[:, 2:3])
            for k in range(3):
                nc.vector.tensor_copy(we[:, T + 2 + k:T + 3 + k], wo[:, T + 2:T + 3])
                nc.vector.tensor_copy(wo[:, k:k + 1], we[:, 2:3])
            for k in range(2):
                nc.vector.tensor_copy(wo[:, T + 3 + k:T + 4 + k], wo[:, T + 2:T + 3])

            osb = opool.tile([128, T], F32, tag="osb")
            for chx in range(NCH):
                off = chx * CH
                op = psum.tile([128, CH], F32, tag="op")
                for c in range(6):
                    nc.tensor.matmul(
                        op[:], lhsT=W(12 + c),
                        rhs=we[:, off + c:off + c + CH],
                        start=(c == 0), stop=False)
                for c in range(6):
                    nc.tensor.matmul(
                        op[:], lhsT=W(18 + c),
                        rhs=wo[:, off + c:off + c + CH],
                        start=False, stop=(c == 5))
                nc.vector.tensor_scalar_add(osb[:, off:off + CH], op[:],
                                            cbs[:, rt:rt + 1])
            nc.sync.dma_start(out_d[rows, :], osb[:])
    nc.compile()
    return nc


def host_inputs(x, alpha, beta, up_filter, down_filter):
    """Build per-core in_maps (numpy only)."""
    uf = np.asarray(up_filter, dtype=np.float64)
    df = np.asarray(down_filter, dtype=np.float64)
    g = 2.0 * uf[[11, 9, 7, 5, 3, 1]]
    go = 2.0 * uf[[10, 8, 6, 4, 2, 0]]
    h = df[[1, 3, 5, 7, 9, 11]]
    g2 = df[[0, 2, 4, 6, 8, 10]]

    eye = np.eye(128, dtype=np.float64)
    blocks = [s * eye for s in list(g) + list(go) + list(h) + list(g2)]
    wmats = np.concatenate(blocks, axis=1).astype(ml_dtypes.bfloat16)  # [128, 24*128]

    Ca = np.exp(np.asarray(alpha, dtype=np.float64)).reshape(C)
    Cb = 1.0 / (np.exp(np.asarray(beta, dtype=np.float64)) + 1e-9).reshape(C)
    ch_of_row = np.arange(RPC) % C
    s1 = (Ca[ch_of_row] / math.pi).reshape(NT, 128).T.astype(np.float32).copy()
    ncbh = (-0.5 * Cb[ch_of_row]).reshape(NT, 128).T.astype(np.float32).copy()
    cbs = (0.5 * Cb[ch_of_row] * df.sum()).reshape(NT, 128).T.astype(np.float32).copy()

    x = np.asarray(x, dtype=np.float32)
    in_maps = []
    for i in range(NCORES):
        shard = np.ascontiguousarray(
            x[i * BPC:(i + 1) * BPC].reshape(RPC, T))
        in_maps.append({"x": shard, "wmats": wmats, "s1": s1,
                        "ncbh": ncbh, "cbs": cbs})
    return in_maps


def run(x, alpha, beta, up_filter, down_filter, trace=False, **run_kwargs):
    if "nc" not in _CACHE:
        _CACHE["nc"] = build_bass()
    nc = _CACHE["nc"]
    in_maps = host_inputs(x, alpha, beta, up_filter, down_filter)
    res = run_bass_kernel_spmd(nc, in_maps, core_ids=list(range(NCORES)),
                               trace=trace, **run_kwargs)
    out = np.empty((B, C, T), dtype=np.float32)
    for i in range(NCORES):
        out[i * BPC:(i + 1) * BPC] = res.results[i]["out"].reshape(BPC, C, T)
    return out, res


def kernel(x, alpha, beta, up_filter, down_filter):
    out, _ = run(x, alpha, beta, up_filter, down_filter, trace=False)
    return out


def bench(x, alpha, beta, up_filter, down_filter, iters=20):
    """Repeat-timing of the compiled 8-core NEFF via PJRT (device-resident
    inputs, no donation). Returns (per_iter_seconds_min, per_iter_seconds_avg).
    NTFF profiling is unavailable in this axon build, so this is the HW
    timing signal: dispatch overhead is amortized/bounded by taking min."""
    import time
    import jax
    from jax.experimental.shard_map import shard_map
    from jax.sharding import Mesh, PartitionSpec, NamedSharding
    from concourse import mybir as _mb
    from concourse.bass2jax import _bass_exec_p, partition_id_tensor, install_neuronx_cc_hook

    install_neuronx_cc_hook()
    if "nc" not in _CACHE:
        _CACHE["nc"] = build_bass()
    nc = _CACHE["nc"]
    in_maps = host_inputs(x, alpha, beta, up_filter, down_filter)

    in_names, out_names, out_avals, zero_outs = [], [], [], []
    partition_name = nc.partition_id_tensor.name if nc.partition_id_tensor else None
    for alloc in nc.m.functions[0].allocations:
        if not isinstance(alloc, _mb.MemoryLocationSet):
            continue
        name = alloc.memorylocations[0].name
        if alloc.kind == "ExternalInput":
            if name != partition_name:
                in_names.append(name)
        elif alloc.kind == "ExternalOutput":
            shape = tuple(alloc.tensor_shape)
            dtype = _mb.dt.np(alloc.dtype)
            out_names.append(name)
            out_avals.append(jax.core.ShapedArray(shape, dtype))
            zero_outs.append(np.zeros(shape, dtype))
    n_params = len(in_names)
    in_names.extend(out_names)
    if partition_name is not None:
        in_names.append(partition_name)

    def _body(*args):
        operands = list(args)
        if partition_name is not None:
            operands.append(partition_id_tensor())
        return tuple(_bass_exec_p.bind(
            *operands, out_avals=tuple(out_avals), in_names=tuple(in_names),
            out_names=tuple(out_names), lowering_input_output_aliases=(),
            sim_require_finite=True, sim_require_nnan=True, nc=nc))

    devices = jax.devices()[:NCORES]
    mesh = Mesh(np.asarray(devices), ("core",))
    nouts = len(out_names)
    in_specs = (PartitionSpec("core"),) * (n_params + nouts)
    out_specs = (PartitionSpec("core"),) * nouts
    fn = jax.jit(shard_map(_body, mesh=mesh, in_specs=in_specs,
                           out_specs=out_specs, check_rep=False),
                 keep_unused=True)
    sh = NamedSharding(mesh, PartitionSpec("core"))
    per_core = [[np.asarray(m[nm]) for nm in in_names[:n_params]] for m in in_maps]
    dev_in = [jax.device_put(
        np.concatenate([per_core[c][i] for c in range(NCORES)], axis=0), sh)
        for i in range(n_params)]
    dev_zero = [jax.device_put(
        np.zeros((NCORES * z.shape[0], *z.shape[1:]), z.dtype), sh)
        for z in zero_outs]

    out = fn(*dev_in, *dev_zero)
    jax.block_until_ready(out)
    times = []
    for _ in range(iters):
        t0 = time.perf_counter()
        out = fn(*dev_in, *dev_zero)
        jax.block_until_ready(out)
        times.append(time.perf_counter() - t0)
    return min(times), sum(times) / len(times)
